# revision 22
# baseline (speedup 1.0000x reference)
"""Trainium2 Bass kernel for CenterDirGroundtruth.

Contract: kernel(instances, centers) -> (mat [8,13,1,768,768] f32,
gt_centers [8,100,2] f32). Data-parallel: batch b -> core b. All per-pixel
compute on device; host only prepares O(100)-sized tables and constants.

Per-core pipeline:
  - exact per-pixel center gather via digit-decomposed one-hot matmuls
    (3x fp16 table pieces accumulated in PSUM -> bit-exact fp32)
  - per-pixel X/Y/R/theta/sin/cos/near/ignore on DVE/Pool/ACT
  - 21x21 separable gaussian blur as banded matmuls (reflect folded into
    the band matrix), global max, normalize -> center mask channel
  - presence bitmask (int32 OR-reduce) -> gt_centers
"""
import numpy as np

import concourse.bass as bass
import concourse.bacc as bacc
import concourse.tile as tile
from concourse import mybir
from concourse.bass_utils import run_bass_kernel_spmd

dt = mybir.dt
op = mybir.AluOpType
AF = mybir.ActivationFunctionType

H = W = 768
NS = 6          # 128-row stripes
CH = 1024       # L2 chunk elems per stack (8 chunks per stripe)
NCORES = 8
PI = 3.14159265358979


def band_ap(dram, stripe):
    # [12, 128, 64] view of rows [128t, 128t+128) of a [768, 768] fp16 DRAM
    # image: stack s = cols [64s, 64s+64), stream = row-major within band.
    return bass.AP(tensor=dram, offset=stripe * 128 * W,
                   ap=[[64, 12], [W, 128], [1, 64]])


def build_program():
    nc = bacc.Bacc("TRN2", target_bir_lowering=False, debug=False,
                   num_devices=NCORES)

    inst_in = nc.dram_tensor("inst", [H, W], dt.int32, kind="ExternalInput")
    tabs_in = nc.dram_tensor("tabs", [120, 6, 120], dt.float16, kind="ExternalInput")
    swp_in = nc.dram_tensor("swp", [128, 2], dt.float32, kind="ExternalInput")
    iota_in = nc.dram_tensor("iota120", [120, 1], dt.float32, kind="ExternalInput")
    ones12_in = nc.dram_tensor("ones12", [12, 120], dt.float16, kind="ExternalInput")
    redl_in = nc.dram_tensor("redl", [120, 8, 96], dt.float32, kind="ExternalInput")
    rows_in = nc.dram_tensor("rowsc", [128, NS], dt.float32, kind="ExternalInput")
    colf_in = nc.dram_tensor("colf", [1, W], dt.float32, kind="ExternalInput")
    gmh_in = nc.dram_tensor("gmh", [128, NS, W], dt.float16, kind="ExternalInput")
    gml_in = nc.dram_tensor("gml", [128, NS, W], dt.float16, kind="ExternalInput")
    ident_in = nc.dram_tensor("ident", [128, 128], dt.float16, kind="ExternalInput")
    shv_in = nc.dram_tensor("shv", [128, 1], dt.int32, kind="ExternalInput")

    a_scr = nc.dram_tensor("a_scr", [H, W], dt.float16, kind="Internal")
    b_scr = nc.dram_tensor("b_scr", [H, W], dt.float16, kind="Internal")
    w4_scr = nc.dram_tensor("w4_scr", [128, 4], dt.int32, kind="Internal")
    wz_scr = nc.dram_tensor("wz_scr", [4, 1], dt.int32, kind="Internal")
    blur_scr = nc.dram_tensor("blur_scr", [H, W], dt.float32, kind="Internal")

    mat_out = nc.dram_tensor("mat", [13, H, W], dt.float32, kind="ExternalOutput")
    gtc_out = nc.dram_tensor("gtc", [100, 2], dt.float32, kind="ExternalOutput")

    with tile.TileContext(nc) as tc:
        _build_body(nc, tc, locals())
    nc.compile()
    return nc


def _build_body(nc, tc, T):
    inst_in, tabs_in, swp_in, iota_in = T["inst_in"], T["tabs_in"], T["swp_in"], T["iota_in"]
    ones12_in, redl_in, rows_in, colf_in = T["ones12_in"], T["redl_in"], T["rows_in"], T["colf_in"]
    gmh_in, gml_in, ident_in, shv_in = T["gmh_in"], T["gml_in"], T["ident_in"], T["shv_in"]
    a_scr, b_scr, w4_scr, wz_scr, blur_scr = T["a_scr"], T["b_scr"], T["w4_scr"], T["wz_scr"], T["blur_scr"]
    mat_out, gtc_out = T["mat_out"], T["gtc_out"]

    import contextlib
    ctx = contextlib.ExitStack()
    with ctx:
        cpool = ctx.enter_context(tc.tile_pool(name="consts", bufs=1))
        keep = ctx.enter_context(tc.tile_pool(name="keep", bufs=1))
        bctx = contextlib.ExitStack()
        l2p = bctx.enter_context(tc.tile_pool(name="l2", bufs=1))
        chp = bctx.enter_context(tc.tile_pool(name="chunks", bufs=2))
        dsp = bctx.enter_context(tc.tile_pool(name="down", bufs=1))
        outp = bctx.enter_context(tc.tile_pool(name="outs", bufs=2))
        vp = bctx.enter_context(tc.tile_pool(name="vload", bufs=2))
        cxp = bctx.enter_context(tc.tile_pool(name="cxcy", bufs=2))

        # ---- constants ----
        iota = cpool.tile([120, 1], dt.float32)
        nc.sync.dma_start(out=iota, in_=iota_in[:, :])
        ones12 = cpool.tile([12, 120], dt.float16)
        nc.sync.dma_start(out=ones12, in_=ones12_in[:, :])
        tabs = cpool.tile([120, 6, 120], dt.float16)
        nc.sync.dma_start(out=tabs, in_=tabs_in[:, :, :])
        redl = cpool.tile([120, 8, 96], dt.float32)
        nc.sync.dma_start(out=redl, in_=redl_in[:, :, :])
        rowsc = cpool.tile([128, NS], dt.float32)
        nc.sync.dma_start(out=rowsc, in_=rows_in[:, :])
        colt = cpool.tile([128, W], dt.float32)
        nc.sync.dma_start(out=colt, in_=bass.AP(tensor=colf_in, offset=0,
                                                ap=[[0, 128], [1, W]]))
        shv = cpool.tile([128, 1], dt.int32)
        nc.sync.dma_start(out=shv, in_=shv_in[:, :])
        swp = cpool.tile([128, 2], dt.float32)
        nc.sync.dma_start(out=swp, in_=swp_in[:, :])
        onesi = cpool.tile([128, W], dt.int32)
        nc.vector.memset(onesi, 1)
        tinyb = cpool.tile([128, 1], dt.float32)
        nc.vector.memset(tinyb, 1e-30)

        # ---- persistent state ----
        macc = keep.tile([128, NS], dt.float32)       # per-stripe sum(v>0)
        bmw = keep.tile([128, 4, NS], dt.int32)      # presence words per stripe
        m_t = [keep.tile([128, W], dt.float16, tag=f"m{t}", name=f"m{t}") for t in range(NS)]
        near_t = [keep.tile([128, W], dt.float16, tag=f"nr{t}", name=f"nr{t}") for t in range(NS)]
        has01 = keep.tile([128, 1], dt.float32)
        bmx = keep.tile([128, NS], dt.float32)
        binv = keep.tile([128, 1], dt.float32)

        # ---- phase A: valid-mask accumulation -> has_inst ----
        for t in range(NS):
            v = vp.tile([128, W], dt.int32, tag="v")
            nc.sync.dma_start(out=v, in_=inst_in[128 * t:128 * (t + 1), :])
            nc.vector.tensor_single_scalar(out=m_t[t], in_=v, scalar=0,
                                           op=op.is_gt)
            nc.vector.tensor_reduce(out=macc[:, t:t + 1], in_=m_t[t],
                                    axis=mybir.AxisListType.X, op=op.add)
        hsum = keep.tile([128, 1], dt.float32)
        nc.vector.tensor_reduce(out=hsum, in_=macc, axis=mybir.AxisListType.X,
                                op=op.add)
        hall = keep.tile([128, 1], dt.float32)
        nc.gpsimd.partition_all_reduce(out_ap=hall[:, :], in_ap=hsum[:, :],
                                       channels=128,
                                       reduce_op=bass.bass_isa.ReduceOp.add)
        nc.vector.tensor_single_scalar(out=has01, in_=hall, scalar=0.0,
                                       op=op.is_gt)

        # ---- phase B: per-stripe gather + per-pixel math ----
        for t in range(NS):
            v = vp.tile([128, W], dt.int32, tag="v")
            nc.sync.dma_start(out=v, in_=inst_in[128 * t:128 * (t + 1), :])

            # digits a = idx//10 via (v*52429)>>19 (exact for v<=99), b = v-10a
            t_i = dsp.tile([128, W], dt.int32, tag="ti")
            nc.vector.tensor_single_scalar(out=t_i, in_=v, scalar=52429,
                                           op=op.mult)
            a_i = dsp.tile([128, W], dt.int32, tag="ai")
            nc.vector.tensor_single_scalar(out=a_i, in_=t_i, scalar=19,
                                           op=op.logical_shift_right)
            a_h = dsp.tile([128, W], dt.float16, tag="ti")
            nc.gpsimd.tensor_single_scalar(out=a_h, in_=a_i, scalar=0, op=op.add)
            b_h = dsp.tile([128, W], dt.float16, tag="bh")
            nc.vector.scalar_tensor_tensor(out=b_h, in0=a_i, scalar=-10, in1=v,
                                           op0=op.mult, op1=op.add)
            sl_t = slice(128 * t, 128 * (t + 1))
            nc.sync.dma_start(out=a_scr[sl_t, :], in_=a_h)
            nc.sync.dma_start(out=b_scr[sl_t, :], in_=b_h)

            # presence bitmask words
            sh = dsp.tile([128, W], dt.int32, tag="sh")
            nc.vector.tensor_single_scalar(out=sh, in_=v, scalar=31,
                                           op=op.bitwise_and)
            w1 = dsp.tile([128, W], dt.int32, tag="w1")
            nc.vector.tensor_tensor(out=w1, in0=onesi, in1=sh,
                                    op=op.logical_shift_left)
            g5 = dsp.tile([128, W], dt.int32, tag="sh")
            nc.vector.tensor_single_scalar(out=g5, in_=v, scalar=5,
                                           op=op.logical_shift_right)
            for c in range(4):
                wd = dsp.tile([128, W], dt.int32, tag="wd")
                nc.vector.scalar_tensor_tensor(out=wd, in0=g5, scalar=c, in1=w1,
                                               op0=op.is_equal, op1=op.mult)
                nc.vector.tensor_reduce(out=bmw[:, c, t:t + 1],
                                        in_=wd, axis=mybir.AxisListType.X,
                                        op=op.bitwise_or)

            # band reload of digits
            a2 = l2p.tile([12, 8192], dt.float16, tag="a2")
            nc.sync.dma_start(out=a2[:, :], in_=band_ap(a_scr, t))
            b2 = l2p.tile([12, 8192], dt.float16, tag="b2")
            nc.sync.dma_start(out=b2[:, :], in_=band_ap(b_scr, t))

            with tc.tile_pool(name="psg", bufs=1, space="PSUM") as psp, \
                 tc.tile_pool(name="psr", bufs=1, space="PSUM") as psr:
                red_cx = psr.tile([96, CH], dt.float32, tag="rcx")
                red_cy = psr.tile([96, CH], dt.float32, tag="rcy")
                for k in range(8):
                    arep = psp.tile([120, CH], dt.float32, tag="repl")
                    brep = psp.tile([120, CH], dt.float32, tag="repl")
                    for h in range(2):
                        hs = slice(h * 512, (h + 1) * 512)
                        gs = slice(k * CH + h * 512, k * CH + (h + 1) * 512)
                        nc.tensor.matmul(out=arep[:, hs], lhsT=ones12[:, :],
                                         rhs=a2[:, gs], start=True, stop=True)
                        nc.tensor.matmul(out=brep[:, hs], lhsT=ones12[:, :],
                                         rhs=b2[:, gs], start=True, stop=True)
                    ea = chp.tile([120, CH], dt.float16, tag="ea")
                    nc.vector.tensor_single_scalar(out=ea, in_=arep[:, :],
                                                   scalar=iota[:, :],
                                                   op=op.is_equal)
                    bsb = chp.tile([120, CH], dt.float16, tag="bsb")
                    nc.scalar.copy(out=bsb, in_=brep[:, :])
                    eb = chp.tile([120, CH], dt.float16, tag="eb")
                    nc.gpsimd.tensor_single_scalar(out=eb, in_=bsb,
                                                   scalar=iota[:, :],
                                                   op=op.is_equal)
                    for chn, red in ((0, red_cx), (1, red_cy)):
                        pp = chp.tile([120, CH], dt.float32, tag="pp")
                        for h in range(2):
                            hs = slice(h * 512, (h + 1) * 512)
                            vv = psp.tile([120, 512], dt.float32, tag="v",
                                          name=f"vv{chn}{h}", bufs=2)
                            for pc in range(3):
                                nc.tensor.matmul(out=vv[:, :],
                                                 lhsT=tabs[:, chn * 3 + pc, :],
                                                 rhs=eb[:, hs],
                                                 start=(pc == 0), stop=(pc == 2))
                            nc.vector.scalar_tensor_tensor(out=pp[:, hs],
                                                           in0=ea[:, hs], scalar=1.0,
                                                           in1=vv[:, :], op0=op.bypass,
                                                           op1=op.mult)
                            nc.tensor.matmul(out=red[:, hs], lhsT=redl[:, k, :],
                                             rhs=pp[:, hs], start=(k == 0),
                                             stop=(k == 7), skip_group_check=True)

                cx = cxp.tile([128, W], dt.float32, tag="cx")
                cy = cxp.tile([128, W], dt.float32, tag="cy")
                for red, dst, nm in ((red_cx, cx, "x"), (red_cy, cy, "y")):
                    ev = cxp.tile([96, CH], dt.float32, tag="ev")
                    nc.scalar.copy(out=ev, in_=red[:, :])
                    for s in range(12):
                        nc.scalar.dma_start(out=dst[:, 64 * s:64 * s + 64],
                                            in_=ev[s::12, :])

            # ---- downstream per-pixel math ----
            X = dsp.tile([128, W], dt.float32, tag="X", bufs=2)
            nc.vector.tensor_single_scalar(out=X, in_=cx, scalar=rowsc[:, t:t + 1],
                                           op=op.subtract)
            Y = dsp.tile([128, W], dt.float32, tag="Y", bufs=2)
            nc.gpsimd.tensor_tensor(out=Y, in0=cy, in1=colt, op=op.subtract)
            X2 = dsp.tile([128, W], dt.float32, tag="X2")
            nc.scalar.activation(out=X2, in_=X, func=AF.Square)
            Y2 = dsp.tile([128, W], dt.float32, tag="Y2")
            nc.scalar.activation(out=Y2, in_=Y, func=AF.Square)
            r2 = dsp.tile([128, W], dt.float32, tag="sh")
            nc.gpsimd.tensor_tensor(out=r2, in0=X2, in1=Y2, op=op.add)
            R = dsp.tile([128, W], dt.float32, tag="Y2")
            nc.scalar.activation(out=R, in_=r2, func=AF.Sqrt, bias=tinyb[:, :])
            rin = dsp.tile([128, W], dt.float32, tag="rin")
            nc.vector.reciprocal(out=rin, in_=R)
            rm = dsp.tile([128, W], dt.float32, tag="rm")
            nc.gpsimd.tensor_tensor(out=rm, in0=rin, in1=m_t[t], op=op.mult)

            o_R = outp.tile([128, W], dt.float32, tag="oA")
            nc.gpsimd.tensor_tensor(out=o_R, in0=R, in1=m_t[t], op=op.mult)
            nc.sync.dma_start(out=mat_out[0, sl_t, :], in_=o_R)
            o_s = outp.tile([128, W], dt.float32, tag="oB")
            nc.gpsimd.tensor_tensor(out=o_s, in0=Y, in1=rm, op=op.mult)
            nc.sync.dma_start(out=mat_out[2, sl_t, :], in_=o_s)
            o_c = outp.tile([128, W], dt.float32, tag="oA")
            nc.vector.tensor_tensor(out=o_c, in0=X, in1=rm, op=op.mult)
            nc.sync.dma_start(out=mat_out[3, sl_t, :], in_=o_c)

            # theta = atan(Y/X) + pi*sign(Y)*(X<0), gated by has_inst
            rx = dsp.tile([128, W], dt.float32, tag="rin")
            nc.vector.reciprocal(out=rx, in_=X)
            q = dsp.tile([128, W], dt.float32, tag="q")
            nc.gpsimd.tensor_tensor(out=q, in0=Y, in1=rx, op=op.mult)
            wat = dsp.tile([128, W], dt.float32, tag="wat")
            nc.scalar.activation(out=wat, in_=q, func=AF.Arctan)
            sg = dsp.tile([128, W], dt.float32, tag="q2")
            nc.scalar.activation(out=sg, in_=Y, func=AF.Sign)
            S2 = dsp.tile([128, W], dt.float32, tag="S2")
            nc.gpsimd.tensor_single_scalar(out=S2, in_=X, scalar=0.0, op=op.is_lt)
            u = dsp.tile([128, W], dt.float32, tag="u")
            nc.vector.scalar_tensor_tensor(out=u, in0=sg, scalar=PI, in1=S2,
                                           op0=op.mult, op1=op.mult)
            th0 = dsp.tile([128, W], dt.float32, tag="q")
            nc.gpsimd.tensor_tensor(out=th0, in0=u, in1=wat, op=op.add)
            o_t = outp.tile([128, W], dt.float32, tag="oB")
            nc.gpsimd.tensor_single_scalar(out=o_t, in_=th0, scalar=has01[:, :],
                                           op=op.mult)
            nc.sync.dma_start(out=mat_out[1, sl_t, :], in_=o_t)

            # near / ignore
            aX = dsp.tile([128, W], dt.float32, tag="X2")
            nc.scalar.activation(out=aX, in_=X, func=AF.Abs)
            aY = dsp.tile([128, W], dt.float32, tag="Y2")
            nc.scalar.activation(out=aY, in_=Y, func=AF.Abs)
            nX = dsp.tile([128, W], dt.float32, tag="nX")
            nc.gpsimd.tensor_single_scalar(out=nX, in_=aX, scalar=3.0, op=op.is_lt)
            nXh = dsp.tile([128, W], dt.float32, tag="nXh")
            nc.vector.tensor_single_scalar(out=nXh, in_=nX, scalar=has01[:, :],
                                           op=op.mult)
            nY = dsp.tile([128, W], dt.float32, tag="nX")
            nc.vector.tensor_single_scalar(out=nY, in_=aY, scalar=3.0, op=op.is_lt)
            nc.vector.tensor_tensor(out=near_t[t], in0=nXh, in1=nY, op=op.mult)
            o_i = outp.tile([128, W], dt.float32, tag="oC")
            nc.vector.tensor_scalar(out=o_i, in0=near_t[t], scalar1=-1.0,
                                    scalar2=1.0, op0=op.mult, op1=op.add)
            nc.sync.dma_start(out=mat_out[4, sl_t, :], in_=o_i)

        # ---- phase C: blur ----
        bctx.close()
        cpool2 = ctx.enter_context(tc.tile_pool(name="cconsts", bufs=1))
        gmh = cpool2.tile([128, NS, W], dt.float16)
        nc.sync.dma_start(out=gmh, in_=gmh_in[:, :, :])
        gml = cpool2.tile([128, NS, W], dt.float16)
        nc.sync.dma_start(out=gml, in_=gml_in[:, :, :])
        ident = cpool2.tile([128, 128], dt.float16)
        nc.sync.dma_start(out=ident, in_=ident_in[:, :])
        bhs_h = cpool2.tile([128, NS, W], dt.float16)
        bhs_l = cpool2.tile([128, NS, W], dt.float16)
        outc = ctx.enter_context(tc.tile_pool(name="outc", bufs=2))
        with tc.tile_pool(name="pst", bufs=2, space="PSUM") as pst:
            nT = cpool2.tile([128, NS, W], dt.float16)
            for t in range(NS):
                for tt in range(NS):
                    tp = pst.tile([128, 128], dt.float16, tag="tp")
                    nc.tensor.transpose(tp[:, :], near_t[t][:, 128 * tt:128 * (tt + 1)],
                                        ident[:, :])
                    nc.scalar.copy(out=nT[:, tt, 128 * t:128 * (t + 1)], in_=tp[:, :])
        with tc.tile_pool(name="psb", bufs=2, space="PSUM") as psb:
            # H-stage: blurH[r, j] = sum_c near[r, c] G[c, j]
            for rt in range(NS):
                bh_ps = psb.tile([128, W], dt.float32, tag="bh")
                for h, hsl in ((0, slice(0, 512)), (1, slice(512, W))):
                    n_acc = 2 * NS
                    i = 0
                    for tcc in range(NS):
                        for gm in (gmh, gml):
                            nc.tensor.matmul(out=bh_ps[:, hsl],
                                             lhsT=nT[:, tcc, 128 * rt:128 * (rt + 1)],
                                             rhs=gm[:, tcc, hsl],
                                             start=(i == 0), stop=(i == n_acc - 1))
                            i += 1
                nc.scalar.copy(out=bhs_h[:, rt, :], in_=bh_ps[:, :])
                nc.vector.tensor_tensor(out=bhs_l[:, rt, :], in0=bh_ps[:, :],
                                        in1=bhs_h[:, rt, :], op=op.subtract)
            # V-stage: blur[i, j] = sum_r G[r, i] blurH[r, j]
            for it in range(NS):
                bl_ps = psb.tile([128, W], dt.float32, tag="bl")
                for h, hsl in ((0, slice(0, 512)), (1, slice(512, W))):
                    terms = []
                    for tr in range(NS):
                        terms += [(gmh, bhs_h, tr), (gml, bhs_h, tr),
                                  (gmh, bhs_l, tr)]
                    for i, (gm, bh, tr) in enumerate(terms):
                        nc.tensor.matmul(out=bl_ps[:, hsl],
                                         lhsT=gm[:, tr, 128 * it:128 * (it + 1)],
                                         rhs=bh[:, tr, hsl],
                                         start=(i == 0), stop=(i == len(terms) - 1))
                nc.vector.tensor_reduce(out=bmx[:, it:it + 1], in_=bl_ps[:, :],
                                        axis=mybir.AxisListType.X, op=op.max)
                blsb = outc.tile([128, W], dt.float32, tag="oD")
                nc.scalar.copy(out=blsb, in_=bl_ps[:, :])
                nc.sync.dma_start(out=blur_scr[128 * it:128 * (it + 1), :], in_=blsb)

        bm1 = keep.tile([128, 1], dt.float32)
        nc.vector.tensor_reduce(out=bm1, in_=bmx, axis=mybir.AxisListType.X,
                                op=op.max)
        bm2 = keep.tile([128, 1], dt.float32)
        nc.gpsimd.partition_all_reduce(out_ap=bm2[:, :], in_ap=bm1[:, :],
                                       channels=128,
                                       reduce_op=bass.bass_isa.ReduceOp.max)
        bm3 = keep.tile([128, 1], dt.float32)
        nc.vector.tensor_single_scalar(out=bm3, in_=bm2, scalar=1e-12, op=op.max)
        nc.vector.reciprocal(out=binv, in_=bm3)
        for it in range(NS):
            bl2 = outc.tile([128, W], dt.float32, tag="oD")
            nc.sync.dma_start(out=bl2, in_=blur_scr[128 * it:128 * (it + 1), :])
            o_m = outc.tile([128, W], dt.float32, tag="oC")
            nc.vector.tensor_single_scalar(out=o_m, in_=bl2, scalar=binv[:, :],
                                           op=op.mult)
            nc.sync.dma_start(out=mat_out[5, 128 * it:128 * (it + 1), :], in_=o_m)

        # ---- phase D: presence -> gt_centers ----
        w4 = keep.tile([128, 4], dt.int32)
        for c in range(4):
            nc.vector.tensor_reduce(out=w4[:, c:c + 1], in_=bmw[:, c, :],
                                    axis=mybir.AxisListType.X, op=op.bitwise_or)
        nc.sync.dma_start(out=w4_scr[:, :], in_=w4)
        w4t = keep.tile([4, 128], dt.int32)
        nc.sync.dma_start(out=w4t[:, :], in_=bass.AP(tensor=w4_scr, offset=0,
                                                     ap=[[1, 4], [4, 128]]))
        wz = keep.tile([4, 1], dt.int32)
        nc.vector.tensor_reduce(out=wz, in_=w4t, axis=mybir.AxisListType.X,
                                op=op.bitwise_or)
        nc.sync.dma_start(out=wz_scr[:, :], in_=wz[:, :])
        w128 = keep.tile([128, 1], dt.int32)
        nc.sync.dma_start(out=w128[:, :], in_=bass.AP(tensor=wz_scr, offset=0,
                                                      ap=[[1, 4], [0, 32]]))
        bits = keep.tile([128, 1], dt.int32)
        nc.vector.tensor_scalar(out=bits, in0=w128, scalar1=shv[:, :], scalar2=1,
                                op0=op.logical_shift_right, op1=op.bitwise_and)
        pres = keep.tile([128, 1], dt.float32)
        nc.gpsimd.tensor_single_scalar(out=pres, in_=bits, scalar=0, op=op.add)
        gtc_t = keep.tile([128, 2], dt.float32)
        nc.vector.tensor_single_scalar(out=gtc_t, in_=swp, scalar=pres[:, :],
                                       op=op.mult)
        nc.sync.dma_start(out=gtc_out[:, :], in_=gtc_t[0:100, :])


def _host_tables(centers_b):
    """Per-core gather tables and swapped-centers table from centers [100, 2]."""
    T = np.zeros((100, 2), np.float64)
    T[1:, 0] = centers_b[:99, 1]   # ch0 = gathered cx (swapped col 1)
    T[1:, 1] = centers_b[:99, 0]   # ch1 = gathered cy
    T[0, :] = -10000.0
    h = T.astype(np.float16).astype(np.float64)
    m = (T - h).astype(np.float16).astype(np.float64)
    l = (T - h - m).astype(np.float16).astype(np.float64)
    assert np.array_equal((h + m + l).astype(np.float32), T.astype(np.float32))
    pieces = np.stack([h, m, l], 0).astype(np.float16)  # [3, 100, 2]

    tabs = np.zeros((120, 6, 120), np.float16)
    blk = np.zeros((10, 6, 10), np.float16)
    for j in range(10):
        for i in range(10):
            k = 10 * i + j
            for chn in range(2):
                for pc in range(3):
                    blk[j, chn * 3 + pc, i] = pieces[pc, k, chn]
    for s in range(12):
        tabs[10 * s:10 * s + 10, :, 10 * s:10 * s + 10] = blk

    swp = np.zeros((128, 2), np.float32)
    swp[1:100, 0] = centers_b[:99, 1]
    swp[1:100, 1] = centers_b[:99, 0]
    return tabs, swp


def _gauss_band():
    """[768, 768] reflect-folded 21-tap gaussian band matrix, fp16 hi+lo."""
    r, sig = 10, 2.0
    x = np.arange(-r, r + 1, dtype=np.float64)
    g = np.exp(-0.5 * (x / sig) ** 2)
    g = (g / g.sum()).astype(np.float32).astype(np.float64)  # match jax f32 kernel
    G = np.zeros((H, H), np.float64)
    for j in range(H):
        for d in range(-r, r + 1):
            srcp = j + d
            if srcp < 0:
                srcp = -srcp
            elif srcp >= H:
                srcp = 2 * H - 2 - srcp
            G[srcp, j] += g[d + r]
    gh = G.astype(np.float16).astype(np.float64)
    gl = (G - gh).astype(np.float16)
    # [128, 6, 768] layouts: row rr = 128*t + p
    ghr = gh.astype(np.float16).reshape(NS, 128, H).transpose(1, 0, 2).copy()
    glr = gl.reshape(NS, 128, H).transpose(1, 0, 2).copy()
    return ghr, glr


_CACHED = {}


def _consts():
    if "c" in _CACHED:
        return _CACHED["c"]
    iota = (np.arange(120) % 10).astype(np.float32).reshape(120, 1)
    ones12 = np.zeros((12, 120), np.float16)
    for s in range(12):
        ones12[s, 10 * s:10 * s + 10] = 1.0
    redl = np.zeros((120, 8, 96), np.float32)
    for k in range(8):
        for s in range(12):
            redl[10 * s:10 * s + 10, k, 12 * k + s] = 1.0
    rowsc = np.zeros((128, NS), np.float32)
    for t in range(NS):
        rowsc[:, t] = 128 * t + np.arange(128)
    colf = np.arange(W, dtype=np.float32).reshape(1, W)
    gmh, gml = _gauss_band()
    ident = np.eye(128, dtype=np.float16)
    shv = (np.arange(128) & 31).astype(np.int32).reshape(128, 1)
    _CACHED["c"] = dict(iota120=iota, ones12=ones12, redl=redl, rowsc=rowsc,
                        colf=colf, gmh=gmh, gml=gml, ident=ident, shv=shv)
    return _CACHED["c"]


def kernel(instances, centers):
    instances = np.asarray(instances)
    centers = np.asarray(centers, dtype=np.float32)
    B = instances.shape[0]
    assert B == NCORES and instances.shape[-2:] == (H, W)

    if "nc" not in _CACHED:
        _CACHED["nc"] = build_program()
    nc = _CACHED["nc"]
    consts = _consts()

    in_maps = []
    for b in range(B):
        tabs, swp = _host_tables(centers[b])
        m = dict(consts)
        m["inst"] = np.ascontiguousarray(instances[b, 0]).astype(np.int32)
        m["tabs"] = tabs
        m["swp"] = swp
        in_maps.append(m)

    res = run_bass_kernel_spmd(nc, in_maps, core_ids=list(range(NCORES)))
    mat = np.stack([res.results[b]["mat"] for b in range(B)])[:, :, None]
    gtc = np.stack([res.results[b]["gtc"] for b in range(B)])
    return mat.astype(np.float32), gtc.astype(np.float32)
